# revision 1
# baseline (speedup 1.0000x reference)
"""Causal self-attention (B=4, T=2048, C=1024, H=16) on 8 trn2 NeuronCores.

Sharding: batch x head-group, zero collectives. Core c handles batch b = c//2
and head-group hg = c%2 (heads hg*8 .. hg*8+8, i.e. half the heads) for ALL
2048 tokens of that batch. Each core computes Q/K/V only for its 8 heads
(no redundant projection work), runs causal attention for those heads, and
produces a PARTIAL output projection out_partial = y_local @ W_proj[hg*512 :
hg*512+512].  The host sums the two partials of each batch (the only
"collective" is a numpy add on the host).

Per-core pipeline (matmul cost = output free-size columns; every matmul PSUM
target sits at a 2KB bank base -- hardware requirement):
  A) QKV projection (bf16): Q^T/K^T feature-major [128, 4 head-pairs, 2048]
     (head-pair d-features packed at partition offsets 0/64); V token-major
     [128 tok, kt, 8 heads, 64+1] with a ones column for the softmax
     denominator.  196,608 PE cols.
  B) Attention over 512-query blocks qb (head-pair mg, 128-key tiles kt):
     S^T[k,q] pairs fill a 2-bank PSUM tile exactly; P^T = exp(0.125 S^T)
     on ScalarE (1024 cols/instr); the 4 diagonal k-tiles of each block
     write their valid columns at the bank base and exp remaps them to the
     aligned pt position, so the causal mask is a single [128,128] tril
     multiply on the one straddled 128-q chunk.  O[q, 64+1] accumulates
     token-major in 8 single-bank passes per block (lhsT = P^T 128-q chunk,
     rhs = V tile, 65 cols); normalization is a per-partition reciprocal +
     tensor_scalar multiply.  S 139,264 + O 70,720 PE cols.
  C) y (token-major bf16) is PE-transposed per [128,128] tile into yT
     feature-major; out_partial^T[c_out, q] = W_proj_slice^T @ yT in bf16.
     8,192 + 65,536 PE cols.
  Total ~480k PE cols ~= 200us at 2.4 GHz.

ScalarE exp (~139k lane-cols ~= 120us + per-instr overhead) dominates phase
B's own PE work, so emission interleaves A's last three token-chunks and
all of C's column-groups between B's (mg, qb) blocks to keep the PE dense
while the exp backlog drains; the first O pass runs one k-tile behind
S/exp (software pipelining) so the PE never waits on ScalarE latency.

CC_PHASES env (timing diagnostics): "A" / "AB" / "ABC" (default) builds a
kernel truncated after that phase, with intermediate tensors DMA'd out.
"""

import os
import sys

import numpy as np

for _p in ("/opt/trn_rl_repo",):
    if os.path.isdir(_p) and _p not in sys.path:
        sys.path.insert(0, _p)

import ml_dtypes

B, T, C, H = 4, 2048, 1024, 16
HD = C // H  # 64
P = 128
CI = C // P  # 8 contraction chunks for QKV
NCORE = 8
QB = 512  # query block for S (fills a PSUM bank per head)
NQB = T // QB  # 4
NKT = T // P  # 16 k-tiles
HL = 8  # local heads per core
CL = HL * HD  # 512 local y features
BF16 = ml_dtypes.bfloat16

_CACHE = {}
LAST_RESULTS = None


def _build():
    """Build + compile the (single, uniform) bass module once."""
    from contextlib import ExitStack

    import concourse.bass as bass  # noqa: F401
    import concourse.mybir as mybir
    from concourse import bacc, masks, tile

    dt = mybir.dt
    f32, bf16, f8 = dt.float32, dt.bfloat16, dt.float8e4
    DR = mybir.MatmulPerfMode.DoubleRow
    EXP = mybir.ActivationFunctionType.Exp

    phases = os.environ.get("CC_PHASES", "ABC")
    repeat = int(os.environ.get("CC_REPEAT", "1"))
    nc = bacc.Bacc(
        "TRN2",
        target_bir_lowering=False,
        debug=False,
        enable_asserts=False,
        num_devices=NCORE,
    )
    xt = nc.dram_tensor("xt", [C, T], bf16, kind="ExternalInput").ap()
    wa = nc.dram_tensor("wa", [C, 3 * CL], bf16, kind="ExternalInput").ap()
    wp = nc.dram_tensor("wp", [CL, C], bf16, kind="ExternalInput").ap()
    mk = nc.dram_tensor("mk", [P, P], bf16, kind="ExternalInput").ap()
    if phases == "ABC":
        out_t = nc.dram_tensor("out_t", [C, T], bf16, kind="ExternalOutput").ap()
    elif phases == "AB":
        out_y = nc.dram_tensor("out_y", [P, NKT * CL], bf16, kind="ExternalOutput").ap()
    else:
        out_k = nc.dram_tensor("out_k", [P, 4 * T], bf16, kind="ExternalOutput").ap()
        out_q = nc.dram_tensor("out_q", [P, 4 * T], bf16, kind="ExternalOutput").ap()
        out_v = nc.dram_tensor(
            "out_v", [P, NKT * HL * (HD + 1)], bf16, kind="ExternalOutput"
        ).ap()

    with tile.TileContext(nc) as tc, ExitStack() as ctx:
      for _rep in range(repeat):
            rep_ctx = ctx if repeat == 1 else ExitStack()
            res = rep_ctx.enter_context(tc.tile_pool(name="res", bufs=1))
            inp = rep_ctx.enter_context(tc.tile_pool(name="inp", bufs=1))
            KT8 = res.tile([P, 2, 4, T], f8, name="KT8")
            QT8 = res.tile([P, 2, 4, T], f8, name="QT8")
            V = res.tile([P, NKT, HL, HD + 1], bf16, name="Vt")
            Y = res.tile([P, NKT, CL], bf16, name="Y")
            yT = res.tile([P, 4, T], bf16, name="yT")
            mask = res.tile([P, P], bf16, name="mask")
            ident = res.tile([P, P], bf16, name="ident")
            wp_sb = res.tile([P, 4, C], bf16, name="wp_sb")
            xt_sb = inp.tile([P, CI, T], bf16, name="xt_sb")
            wa_sb = inp.tile([P, CI, 3 * CL], bf16, name="wa_sb")

            # Input DMAs, ordered so the first A matmuls start early:
            # Q-weights + token chunk 0 first, then K/V weights, the rest.
            wa_r = wa.rearrange("(o p) f -> p o f", p=P)
            xt_r = xt.rearrange("(o p) t -> p o t", p=P)
            nc.sync.dma_start(wa_sb[:, :, :P], wa_r[:, :, :P])
            nc.sync.dma_start(xt_sb[:, :, :256], xt_r[:, :, :256])
            nc.sync.dma_start(xt_sb[:, :, 256:512], xt_r[:, :, 256:512])
            nc.sync.dma_start(mask, mk)
            nc.sync.dma_start(wa_sb[:, :, P:CL], wa_r[:, :, P:CL])
            nc.sync.dma_start(wa_sb[:, :, CL:2 * CL], wa_r[:, :, CL:2 * CL])
            nc.sync.dma_start(wa_sb[:, :, 2 * CL:], wa_r[:, :, 2 * CL:])
            nc.sync.dma_start(xt_sb[:, :, 512:1024], xt_r[:, :, 512:1024])
            nc.sync.dma_start(xt_sb[:, :, 1024:], xt_r[:, :, 1024:])
            nc.sync.dma_start(wp_sb, wp.rearrange("(o p) f -> p o f", p=P))
            masks.make_identity(nc, ident)
            nc.gpsimd.memset(V[:, :, :, HD:], 1.0)

            psS = rep_ctx.enter_context(
                tc.tile_pool(name="psS", bufs=2, space="PSUM")
            )
            psO = rep_ctx.enter_context(
                tc.tile_pool(name="psO", bufs=2, space="PSUM")
            )
            psM = rep_ctx.enter_context(
                tc.tile_pool(name="psM", bufs=2, space="PSUM")
            )
            pP = rep_ctx.enter_context(tc.tile_pool(name="pP", bufs=20))
            stg = rep_ctx.enter_context(tc.tile_pool(name="stg", bufs=4))
            std = rep_ctx.enter_context(
                tc.tile_pool(name="std", bufs=4, space="DRAM")
            )
            stage = {}
            pR = rep_ctx.enter_context(tc.tile_pool(name="pR", bufs=8))
            osb = rep_ctx.enter_context(tc.tile_pool(name="osb", bufs=3))

            def gen_a(tb, g, filler=False):
                """One A unit: feature group g (0-3 Q, 4-7 K, 8-11 V) for
                token chunk tb (512 tokens).  Yields after each matmul so
                the scheduler can interleave the S stream (which feeds the
                ScalarE exp pipeline) at instruction granularity."""
                ps = psM.tile([P, 512], f32, name="psM_t")
                if g < 8:
                    for ci in range(CI):
                        nc.tensor.matmul(
                            ps,
                            lhsT=wa_sb[:, ci, g * P:(g + 1) * P],
                            rhs=xt_sb[:, ci, tb * 512:(tb + 1) * 512],
                            start=(ci == 0),
                            stop=(ci == CI - 1),
                        )
                        yield
                    # cast to fp8 into the per-(tensor, tb) staging tile; a
                    # fold DMA then repacks partitions 64h'+32dh+dl ->
                    # (32h'+dl, dh) for the DoubleRow S layout.
                    key = (tb, g < 4)
                    if key not in stage:
                        stage[key] = stg.tile([P, 4, 512], f8, name="stg_t")
                    st = stage.pop(key) if g % 4 == 3 else stage[key]
                    eng = nc.vector.tensor_copy if filler else nc.scalar.copy
                    eng(st[:, g % 4], ps)
                    if g % 4 == 3:
                        # partition fold via DRAM (SBUF free dims cannot
                        # cross partitions): st -> scratch -> DoubleRow tile
                        dst = QT8 if g < 4 else KT8
                        dr = std.tile([P, 4 * 512], f8, name="std_t")
                        nc.sync.dma_start(dr, st.rearrange("p m t -> p (m t)"))
                        nc.sync.dma_start(
                            dst[:64, :, :, tb * 512:(tb + 1) * 512],
                            dr.rearrange(
                                "(a c b) (m t) -> (a c) b m t",
                                a=2, c=32, b=2, t=512,
                            ),
                        )
                else:
                    kt = tb * 4 + (g - 8)
                    for ci in range(CI):
                        nc.tensor.matmul(
                            ps,
                            lhsT=xt_sb[:, ci, kt * P:(kt + 1) * P],
                            rhs=wa_sb[:, ci, 2 * CL:3 * CL],
                            start=(ci == 0),
                            stop=(ci == CI - 1),
                        )
                        yield
                    nc.vector.tensor_copy(
                        V[:, kt, :, :HD], ps.rearrange("p (h d) -> p h d", d=HD)
                    )

            def emit_a(tb, g, filler=False):
                for _ in gen_a(tb, g, filler):
                    pass

            def gen_c(co, q0, w=512):
                """One C unit: out_partial^T rows [co*128,(co+1)*128) for
                query columns [q0, q0+w).  Yields after each matmul."""
                ps = psM.tile([P, 512], f32, name="psM_t")
                for ci in range(4):
                    nc.tensor.matmul(
                        ps[:, :w],
                        lhsT=wp_sb[:, ci, co * P:(co + 1) * P],
                        rhs=yT[:, ci, q0:q0 + w],
                        start=(ci == 0),
                        stop=(ci == 3),
                    )
                    yield
                o_sb = osb.tile([P, 512], bf16, name="o_sb")
                nc.vector.tensor_copy(o_sb[:, :w], ps[:, :w])
                nc.sync.dma_start(out_r[:, co, q0:q0 + w], o_sb[:, :w])

            def emit_c(co, q0, w=512):
                for _ in gen_c(co, q0, w):
                    pass

            def emit_t(qj):
                """Transpose the [128 q, 512 c] row qj of Y into yT."""
                ps = psM.tile([P, 512], f32, name="psM_t")
                pstb = ps.bitcast(bf16)
                for g in range(4):
                    nc.tensor.matmul(
                        pstb[:, g * P:(g + 1) * P],
                        Y[:, qj, g * P:(g + 1) * P],
                        ident,
                        is_transpose=True,
                    )
                nc.vector.tensor_copy(
                    yT[:, :, qj * P:(qj + 1) * P],
                    pstb[:, :512].rearrange("p (g i) -> p g i", g=4),
                )

            def emit_o(mg, qb, kt, pt, o_acc, hh, half):
                """One O accumulation step: P^T chunk (128 q) @ V tile."""
                if kt > 4 * qb + half:  # causally empty for this q chunk
                    return
                nc.tensor.matmul(
                    o_acc[:, :HD + 1],
                    lhsT=pt[:, hh, half * P:(half + 1) * P],
                    rhs=V[:, kt, 2 * mg + hh, :],
                    start=(kt == 0),
                    stop=(kt == 4 * qb + half),
                )

            def emit_norm(mg, qb, hh, half, o_acc):
                qj = 4 * qb + half
                r = pR.tile([P, 1], f32, name="r_sb")
                nc.vector.reciprocal(r, o_acc[:, HD:HD + 1])
                nc.vector.tensor_scalar_mul(
                    Y[:, qj, (2 * mg + hh) * HD:(2 * mg + hh + 1) * HD],
                    o_acc[:, :HD],
                    r,
                )

            def emit_b_block(mg, qb, drain, hooks={}):
                """S/exp for head-pair mg, 512-query block qb; O in eight
                (hh, half) passes, each a single bank-base PSUM accumulator
                from a double-buffered pool so normalization never stalls
                the PE.  `fillers` run mid-block to cover the ScalarE exp
                backlog.

                Diagonal k-tile j (kt = 4qb+j) computes only q columns
                128j..512; the S matmul writes them at its bank base and
                exp remaps them to the aligned pt position, after which the
                single straddled chunk (half == j) gets the tril mask."""
                nkt = 4 * qb + 4
                o_acc = psO.tile([P, 512], f32, name="o_acc")
                pts = []
                for kt in range(nkt):
                    j = kt - 4 * qb  # >= 0 on the 4 diagonal k-tiles
                    pt = pP.tile([P, 2, QB], bf16, name="pt")
                    s2 = psS.tile([P, 2, 512], f32, name="s2")
                    if j <= 0:
                        for hh in range(2):
                            hp = hh * 32
                            nc.tensor.matmul(
                                s2[:, hh, :],
                                lhsT=KT8[hp:hp + 32, :, mg, kt * P:(kt + 1) * P],
                                rhs=QT8[hp:hp + 32, :, mg, qb * QB:(qb + 1) * QB],
                                start=True,
                                stop=True,
                                perf_mode=DR,
                            )
                        nc.scalar.activation(pt, s2, EXP, scale=0.125)
                    else:
                        w = QB - j * P  # valid q columns
                        for hh in range(2):
                            hp = hh * 32
                            nc.tensor.matmul(
                                s2[:, hh, :w],
                                lhsT=KT8[hp:hp + 32, :, mg, kt * P:(kt + 1) * P],
                                rhs=QT8[hp:hp + 32, :, mg,
                                        qb * QB + j * P:(qb + 1) * QB],
                                start=True,
                                stop=True,
                                perf_mode=DR,
                            )
                        nc.scalar.activation(
                            pt[:, :, j * P:], s2[:, :, :w], EXP, scale=0.125
                        )
                    if j >= 0:  # mask the straddled 128-q chunk (half == j)
                        for hh in range(2):
                            nc.vector.tensor_mul(
                                pt[:, hh, j * P:(j + 1) * P],
                                pt[:, hh, j * P:(j + 1) * P],
                                mask,
                            )
                    pts.append(pt)
                    if kt >= 1:  # pass (hh=0, half=0), one k-tile behind
                        emit_o(mg, qb, kt - 1, pts[kt - 1], o_acc, 0, 0)
                emit_norm(mg, qb, 0, 0, o_acc)
                o_acc = psO.tile([P, 512], f32, name="o_acc")
                for kt in range(nkt):
                    emit_o(mg, qb, kt, pts[kt], o_acc, 1, 0)
                emit_norm(mg, qb, 1, 0, o_acc)
                budget = (3, 3, 5, 6)[qb]
                drain((budget + 1) // 2)
                for half in range(1, 4):
                    for hh in range(2):
                        o_acc = psO.tile([P, 512], f32, name="o_acc")
                        for kt in range(nkt):
                            emit_o(mg, qb, kt, pts[kt], o_acc, hh, half)
                        emit_norm(mg, qb, hh, half, o_acc)
                    for hook in hooks.get(half, ()):
                        hook()
                    if half == 1:
                        drain(budget // 2)

            # -------- Phase A prelude: token chunk 0 only; the rest of A
            # and all of C are interleaved into phase B as PE fillers. ------
            ntb_pre = 4 if phases == "A" else 1
            for tb in range(ntb_pre):
                gs = range(12)
                if tb == 0 and phases != "A":
                    # first unit split in half so the opening matmul starts
                    # after ~1.3MB of input DMA instead of 2.3MB
                    key = (0, True)
                    stage[key] = stg.tile([P, 4, 512], f8, name="stg_t")
                    for th in range(2):
                        ps = psM.tile([P, 512], f32, name="psM_t")
                        for ci in range(CI):
                            nc.tensor.matmul(
                                ps[:, :256],
                                lhsT=wa_sb[:, ci, :P],
                                rhs=xt_sb[:, ci, th * 256:(th + 1) * 256],
                                start=(ci == 0),
                                stop=(ci == CI - 1),
                            )
                        nc.scalar.copy(
                            stage[key][:, 0, th * 256:(th + 1) * 256],
                            ps[:, :256],
                        )
                    gs = range(1, 12)
                for g in gs:
                    emit_a(tb, g)

            if phases == "A":
                nc.sync.dma_start(out_k, KT8.bitcast(bf16).rearrange("p a b c -> p (a b c)")[:, :4 * T])
                nc.sync.dma_start(out_q, QT8.bitcast(bf16).rearrange("p a b c -> p (a b c)")[:, :4 * T])
                nc.sync.dma_start(out_v, V.rearrange("p a b c -> p (a b c)"))
            else:
                if phases == "ABC":
                    out_r = out_t.rearrange("(o p) q -> p o q", p=P)

                # Global bulk-work queue (A token-chunks tb1-3, C column
                # groups), interleaved with the S stream at INSTRUCTION
                # granularity: exp (ScalarE) is fed by the S matmuls, so any
                # long PE filler between S emissions starves it.  ~4 bulk
                # matmuls (~850ns) per k-tile matches the per-k-tile gap
                # between the exp cost (~950ns) and the S+O PE cost.
                fillq = []
                ready = {("a", 1), ("a", 2), ("a", 3)}
                for tb in (1, 2, 3):
                    for g in range(12):
                        fillq.append(
                            (("a", tb),
                             lambda tb=tb, g=g: gen_a(tb, g, filler=True))
                        )
                if phases == "ABC":
                    for qch in (0, 1, 2):
                        for co in range(8):
                            fillq.append(
                                (("c", qch),
                                 lambda qch=qch, co=co: gen_c(co, qch * 512))
                            )
                cur = {"gen": None}

                def bulk_step(n, tags=None):
                    steps = 0
                    while steps < n:
                        if cur["gen"] is None:
                            for i, (tag, fn) in enumerate(fillq):
                                if tag in ready and (tags is None or tag in tags):
                                    fillq.pop(i)
                                    cur["gen"] = fn()
                                    break
                            else:
                                return
                        try:
                            next(cur["gen"])
                            steps += 1
                        except StopIteration:
                            cur["gen"] = None

                def drain(n, tags=None):
                    if cur["gen"] is not None:
                        for _ in cur["gen"]:
                            pass
                        cur["gen"] = None
                    done = 0
                    while fillq and (n is None or done < n):
                        for i, (tag, fn) in enumerate(fillq):
                            if tag in ready and (tags is None or tag in tags):
                                fillq.pop(i)
                                for _ in fn():
                                    pass
                                done += 1
                                break
                        else:
                            break

                hooks_last = {}
                if phases == "ABC":
                    # Last block: transpose qj12-13 + emit their C columns as
                    # soon as their normalizes land, shortening the tail.
                    hooks_last[1] = (
                        [lambda: emit_t(12), lambda: emit_t(13)]
                        + [lambda co=co: emit_c(co, 1536, 256) for co in range(8)]
                    )

                for qb in range(NQB):
                    if qb:
                        drain(None, tags={("a", tb) for tb in range(1, qb + 1)})
                    for mg in range(4):
                        hk = hooks_last if (qb == 3 and mg == 3) else {}
                        emit_b_block(mg, qb, drain, hk)
                    if qb < 3:
                        for half in range(4):
                            emit_t(4 * qb + half)
                        ready.add(("c", qb))
                drain(None)

                if phases == "AB":
                    for half in range(4):
                        emit_t(12 + half)
                    nc.sync.dma_start(out_y, Y.rearrange("p a b -> p (a b)"))
                else:
                    emit_t(14)
                    emit_t(15)
                    for co in range(8):
                        emit_c(co, 1792, 256)

            if repeat != 1:
                rep_ctx.close()

    nc.compile()
    return nc


def _prep_inputs(x, W_attn, W_proj):
    """Host-side shard/layout prep. Pure data movement + dtype casts."""
    x = np.asarray(x, dtype=np.float32)
    W_attn = np.asarray(W_attn, dtype=np.float32)
    W_proj = np.asarray(W_proj, dtype=np.float32)
    # Causal tril mask [k within tile, q within 128-chunk]: valid iff
    # kk <= ii.  Applied only to the straddled chunk of diagonal k-tiles.
    kk = np.arange(P)[:, None]
    ii = np.arange(P)[None, :]
    mask = (kk <= ii).astype(np.float32).astype(BF16)

    in_maps = []
    for c in range(NCORE):
        b, hg = c // 2, c % 2
        xt_bf = np.ascontiguousarray(x[b].T).astype(BF16)
        sl = slice(hg * CL, (hg + 1) * CL)
        # Q/K columns permuted within each head so the fp8 staging tile's
        # partition order is (head parity, d%32, d//32) -- then the fold
        # DMA into the DoubleRow layout is a plain adjacent rearrange.
        perm = np.arange(CL).reshape(HL, HD)
        qperm = np.arange(HD) // 2 + 32 * (np.arange(HD) % 2)
        perm = (perm[:, qperm]).reshape(CL)
        wq = W_attn[:, sl][:, perm]
        wk = W_attn[:, C:][:, sl][:, perm]
        wa_bf = np.ascontiguousarray(
            np.concatenate([wq, wk, W_attn[:, 2 * C:][:, sl]], axis=1)
        ).astype(BF16)
        wp_bf = np.ascontiguousarray(W_proj[sl, :]).astype(BF16)
        in_maps.append({"xt": xt_bf, "wa": wa_bf, "wp": wp_bf, "mk": mask})
    return in_maps


def kernel(x, W_attn, W_proj):
    global LAST_RESULTS
    from concourse.bass_utils import run_bass_kernel_spmd

    if "nc" not in _CACHE:
        _CACHE["nc"] = _build()
    nc = _CACHE["nc"]

    in_maps = _prep_inputs(x, W_attn, W_proj)
    trace = os.environ.get("CC_TRACE", "0") == "1"
    res = run_bass_kernel_spmd(nc, in_maps, core_ids=list(range(NCORE)), trace=trace)
    LAST_RESULTS = res

    out = np.empty((B, T, C), dtype=np.float32)
    for b in range(B):
        p0 = res.results[2 * b]["out_t"].astype(np.float32)
        p1 = res.results[2 * b + 1]["out_t"].astype(np.float32)
        out[b] = (p0 + p1).T
    return out

